# revision 19
# baseline (speedup 1.0000x reference)
"""Brownian-bridge criterion loss on 8 Trainium2 NeuronCores.

Two launches, bf16 matmul operands, group-sharded negatives (no
cross-core exchange, no collectives):

  Host (indexing only): sort sequences by bridge pivot; core k owns the
  200 sorted cur sequences [200k, 200k+200) plus 200 other sequences.
  Rows are laid out t-major: cur rows t*200+s (t=0..15), oth rows
  3200+(t-1)*200+s (t=1..14 only -- head/tail of `other` are never
  used). 6000 rows padded to 6016 = 47 partition tiles.

  Launch A (per core): project its 6016 rows with W.  Two row-tiles
  share one full PSUM bank (2 accumulating matmuls per tile, xt
  stationary, W moving; no bias matmul -- the bias is added during
  PSUM evacuation by a single 512-wide DVE scalar_tensor_tensor
  (ys = psum + b_rep) straight to bf16).  Per-tile Square+row-sum
  reads the bf16 ys (alternating DVE stt-accum / ACT Square-accum),
  batched sqrt (ACT) + reciprocal (DVE), then an in-place per-tile
  rescale (alternating ACT Copy-with-scale / GpSimd tensor_scalar).
  Pool rows (t=1..14 of both cur and oth) go to DRAM as bf16 via the
  two hardware-DGE engines (sync/scalar) only.  Epilogue (emitted
  between the cur and oth tile segments so it overlaps) reads g0/g2
  from SBUF slices, gathers g1 by pivot via indirect DMA, and emits
  per-sequence c0, c1, s(self-dist), score as f32.

  Host reshard (indexing only): scatter a^T into a pivot-grouped
  padded A matrix with per-group capacity ceil(count/128)*128 --
  ~15 M-tiles total for uniform pivots instead of a fixed 28.  Tiles
  are paired onto cores in group order (a group's two tiles land on
  one core), so each core runs MTB=2 M-tiles and needs at most two
  3200-column pivot pools.  Pad slots carry c0=c1=0, s=-1e4,
  score=+1e4 and contribute exactly zero.

  Launch B (per core): MTB M-tiles x 7 psum chunks x 2 matmuls ->
  cross [128,512] PSUM, Max8 per chunk + a merge Max8 -> top-8 over
  all 3200 negatives per slot; the affine dist = c1*cross + c0
  (monotone, so top-k commutes), numer = exp(s), the self-exclusion
  identity deno = numer + sum_{j<=5} exp(v_j) - exp(max(v_5, s)), the
  softplus head-tail term (single Softplus activation), and per-core
  partial sums (scaled 1/n) via a ones-matmul.  The host adds the 8
  partial pairs (the unshard).

The instruction stream shape depends only on MTB (tiles per core in
launch B); bridge contents select index/input tensors and the host
packing.  NEFFs are cached per MTB.
"""

import sys

sys.path.insert(0, "/opt/trn_rl_repo")

import numpy as np
import ml_dtypes

import concourse.bacc as bacc
import concourse.bass as bass
import concourse.mybir as mybir
import concourse.tile as tile
from concourse.bass_utils import run_bass_kernel_spmd

F32 = mybir.dt.float32
BF16 = mybir.dt.bfloat16
I32 = mybir.dt.int32
AF = mybir.ActivationFunctionType
OP = mybir.AluOpType
NPBF = ml_dtypes.bfloat16
NPF8 = ml_dtypes.float8_e4m3
FP8 = mybir.dt.float8e4

BS, T, Q, HID, PROJ = 16, 16, 100, 256, 256
NSEQ = BS * Q              # 1600 positive sequences
NCORES = 8
SPC = NSEQ // NCORES       # 200 cur sequences per core
CUR_ROWS = T * SPC         # 3200 t-major cur rows
OTH_ROWS = (T - 2) * SPC   # 2800 t-major oth rows (t=1..14 only)
ROWS = CUR_ROWS + OTH_ROWS  # 6000
NT = 47                    # partition tiles (47*128 = 6016)
RPAD = NT * 128
POOL_CUR = (T - 2) * SPC   # 2800 pool rows from cur (t=1..14)
POOL_ROWS = 2 * POOL_CUR   # 5600 emb rows in DRAM
NG = T - 2                 # 14 pivot groups
DELTA = 0.3
GRP = 4                    # tiles per rsqrt batch in launch A


def _build_la():
    nc = bacc.Bacc("TRN2", target_bir_lowering=False, debug=False,
                   num_devices=NCORES)
    xt_in = nc.declare_dram_parameter("xt_in", [HID, RPAD], BF16,
                                      isOutput=False)
    w_in = nc.declare_dram_parameter("w_in", [HID, PROJ], BF16, isOutput=False)
    b2_in = nc.declare_dram_parameter("b2_in", [128, 4, PROJ], BF16,
                                      isOutput=False)
    br_in = nc.declare_dram_parameter("br_in", [SPC, 3], I32, isOutput=False)
    g1idx = nc.declare_dram_parameter("g1idx", [SPC, 1], I32, isOutput=False)

    emb = nc.declare_dram_parameter("emb", [POOL_ROWS, PROJ], BF16,
                                    isOutput=True)
    a_out = nc.declare_dram_parameter("a_out", [SPC, PROJ], BF16,
                                      isOutput=True)
    sc_out = nc.declare_dram_parameter("sc_out", [SPC, 4], F32, isOutput=True)
    # sc_out cols: 0=c0, 1=c1, 2=s(self dist), 3=score

    # xt loads split into tile-aligned quarters so matmuls can start as
    # soon as the first quarter lands
    QT = (4, 4, 8, 12, 12, 7)
    with tile.TileContext(nc) as tc:
        with (
            tc.tile_pool(name="singles", bufs=1) as singles,
            tc.tile_pool(name="work", bufs=4) as work,
            tc.tile_pool(name="ework", bufs=2) as ework,
            tc.tile_pool(name="psum", bufs=4, space="PSUM") as psum_pool,
        ):
            engs = (nc.sync, nc.scalar)
            qoff = [0]
            for n in QT:
                qoff.append(qoff[-1] + n)

            # priority order: first quarter of xt, then W, then bias,
            # then the remaining xt quarters; big early transfers are
            # split into 128-col pieces so each lands on its own queue
            xt_sb = [[None] * len(QT) for _ in range(2)]
            for kt in range(2):
                t_x = singles.tile([128, QT[0] * 128], BF16, tag=f"xt{kt}_0")
                for r in range(2):
                    engs[kt].dma_start(
                        out=t_x[:, r * 256:(r + 1) * 256],
                        in_=xt_in[kt * 128:(kt + 1) * 128,
                                  r * 256:(r + 1) * 256])
                xt_sb[kt][0] = t_x
            w_sb = []
            for kt in range(2):
                t_w = singles.tile([128, PROJ], BF16, tag=f"w{kt}")
                engs[kt].dma_start(out=t_w,
                                   in_=w_in[kt * 128:(kt + 1) * 128, :])
                w_sb.append(t_w)
            b2_sb = singles.tile([128, 4, PROJ], BF16, tag="bias2")
            for si in range(4):
                engs[si % 2].dma_start(out=b2_sb[:, si, :],
                                       in_=b2_in[:, si, :])
            bi_t = []
            idx_t = []
            for ti, (s0, psz) in enumerate(((0, 128), (128, 72))):
                bi = ework.tile([128, 3], I32, tag="bi")
                nc.gpsimd.dma_start(out=bi[:psz], in_=br_in[s0:s0 + psz, :])
                bi_t.append(bi)
                idx = ework.tile([128, 1], I32, tag="idx")
                nc.gpsimd.dma_start(out=idx[:psz], in_=g1idx[s0:s0 + psz, :])
                idx_t.append(idx)

            for qi in range(1, len(QT)):
                for kt in range(2):
                    t_x = singles.tile([128, QT[qi] * 128], BF16,
                                       tag=f"xt{kt}_{qi}")
                    engs[kt].dma_start(
                        out=t_x,
                        in_=xt_in[kt * 128:(kt + 1) * 128,
                                  qoff[qi] * 128:qoff[qi + 1] * 128])
                    xt_sb[kt][qi] = t_x

            def xt_slice(kt, m):
                qi = 0
                while m >= qoff[qi + 1]:
                    qi += 1
                r = m - qoff[qi]
                return xt_sb[kt][qi][:, r * 128:(r + 1) * 128]

            ss_all = singles.tile([128, NT + 1], F32, tag="ss")
            sn_all = singles.tile([128, NT + 1], F32, tag="sn")
            rs_all = singles.tile([128, NT + 1], F32, tag="rs")
            bn_all = singles.tile([128, NT, 6], F32, tag="bn")
            ys_all = singles.tile([128, NT, PROJ], BF16, tag="ys")

            # emb DRAM row for local row r (t-major):
            #   cur pool rows 200..2999   -> r - 200
            #   oth pool rows 3200..5999  -> r - 400
            def emb_ranges(m):
                lo, hi = m * 128, (m + 1) * 128
                out = []
                a, b2 = max(lo, SPC), min(hi, CUR_ROWS - SPC)
                if a < b2:
                    out.append((a - lo, b2 - lo, a - SPC))
                a, b2 = max(lo, CUR_ROWS), min(hi, ROWS)
                if a < b2:
                    out.append((a - lo, b2 - lo, a - 2 * SPC))
                return out

            out_eng = [0]
            pend = []             # rescale batches skewed one quad behind

            def rescale_batch(sl, tail=False):
                # reciprocal + in-place per-tile rescale on DVE (feeds a
                # quad emitted earlier, so no stall) + emb DMA out; the
                # flush batches issue from sync only (gpsimd descriptor
                # generation is 650ns of engine time and would extend the
                # tail)
                nc.vector.reciprocal(out=rs_all[:, sl], in_=sn_all[:, sl])
                for mm in range(sl.start, sl.stop):
                    nc.vector.tensor_scalar(
                        out=ys_all[:, mm, :], in0=ys_all[:, mm, :],
                        scalar1=rs_all[:, mm:mm + 1], scalar2=None,
                        op0=OP.mult)
                    for (p0, p1, dst) in emb_ranges(mm):
                        eng = (nc.sync if (tail or out_eng[0] % 2 == 0)
                               else nc.gpsimd)
                        out_eng[0] += 1
                        eng.dma_start(
                            out=emb[dst:dst + (p1 - p0), :],
                            in_=ys_all[p0:p1, mm, :])

            # quads whose row-sums run on DVE bn_stats instead of ACT
            # Square+accum -- sized to balance the two engines
            BN_QUADS = (8, 33)

            def quad_step(m0, mend):
                # up to 4 row-tiles across two PSUM banks; bias is added
                # during a single wide evacuation stt (psum + b_rep)
                L = mend - m0
                ps = psum_pool.tile([128, 4, PROJ], F32,
                                    name=f"psq{m0}", tag="ps")
                for si in range(L):
                    for kt in range(2):
                        nc.tensor.matmul(
                            out=ps[:, si, :],
                            lhsT=xt_slice(kt, m0 + si),
                            rhs=w_sb[kt],
                            start=(kt == 0),
                            stop=(kt == 1),
                        )
                nc.vector.scalar_tensor_tensor(
                    out=ys_all[:, m0:mend, :], in0=ps[:, 0:L, :],
                    scalar=1.0, in1=b2_sb[:, 0:L, :],
                    op0=OP.mult, op1=OP.add)
                sl = slice(m0, mend)
                if m0 in BN_QUADS:
                    # DVE path: bn_stats per tile + a 5-op batched fixup
                    # ss = (cvar_e + cvar_o) + 128*(mu_e^2 + mu_o^2)
                    for m in range(m0, mend):
                        nc.vector.bn_stats(out=bn_all[:, m, :],
                                           in_=ys_all[:, m, :])
                    w1 = ework.tile([128, GRP, 1], F32, tag="fx1")
                    w2 = ework.tile([128, GRP, 1], F32, tag="fx2")
                    nc.vector.tensor_tensor(
                        out=w1[:, :L, :], in0=bn_all[:, sl, 1:2],
                        in1=bn_all[:, sl, 1:2], op=OP.mult)
                    nc.vector.tensor_tensor(
                        out=w2[:, :L, :], in0=bn_all[:, sl, 4:5],
                        in1=bn_all[:, sl, 4:5], op=OP.mult)
                    nc.vector.tensor_tensor(
                        out=w1[:, :L, :], in0=w1[:, :L, :],
                        in1=w2[:, :L, :], op=OP.add)
                    nc.vector.tensor_tensor(
                        out=w2[:, :L, :], in0=bn_all[:, sl, 2:3],
                        in1=bn_all[:, sl, 5:6], op=OP.add)
                    nc.vector.scalar_tensor_tensor(
                        out=ss_all[:, sl].unsqueeze(-1), in0=w1[:, :L, :],
                        scalar=128.0, in1=w2[:, :L, :],
                        op0=OP.mult, op1=OP.add)
                else:
                    # ACT path: unary Square + row-sum accumulator
                    for m in range(m0, mend):
                        sq = work.tile([128, PROJ], BF16, tag="sqa")
                        nc.scalar.activation(out=sq, in_=ys_all[:, m, :],
                                             func=AF.Square,
                                             accum_out=ss_all[:, m:m + 1])
                nc.scalar.activation(out=sn_all[:, sl], in_=ss_all[:, sl],
                                     func=AF.Sqrt)
                pend.append(sl)
                if len(pend) > 1:
                    rescale_batch(pend.pop(0))

            CUR_NT = 25           # tiles 0..24 cover all cur rows
            for p in range(0, CUR_NT, 4):
                quad_step(p, min(p + 4, CUR_NT))
            while pend:
                rescale_batch(pend.pop(0))

            # ---- epilogue inputs: prefetch g2 slices + pivot gather so
            # they overlap the oth tiles; the scalar math runs at the end
            # t-major row positions: g0 = rows 0..199 (tiles 0, 1),
            # g2 = rows 3000..3199 (tile 23 p56.., tile 24).
            g2a = singles.tile([128, PROJ], BF16, tag="g2a")
            g2b = singles.tile([128, PROJ], BF16, tag="g2b")
            nc.sync.dma_start(out=g2a[0:72], in_=ys_all[56:128, 23, :])
            nc.sync.dma_start(out=g2a[72:128], in_=ys_all[0:56, 24, :])
            nc.sync.dma_start(out=g2b[0:72], in_=ys_all[56:128, 24, :])

            g1_t = []
            for ti, (s0, psz) in enumerate(((0, 128), (128, 72))):
                g1t = ework.tile([128, PROJ], BF16, tag="g1")
                nc.gpsimd.indirect_dma_start(
                    out=g1t[:psz],
                    out_offset=None,
                    in_=emb[:, :],
                    in_offset=bass.IndirectOffsetOnAxis(
                        ap=idx_t[ti][:psz, :1], axis=0),
                )
                g1_t.append(g1t)

            # ---- oth tiles ----
            for p in range(CUR_NT, NT, 4):
                quad_step(p, min(p + 4, NT))
            while pend:
                rescale_batch(pend.pop(0), tail=True)

            # ---- epilogue math: pure tail, nothing downstream waits ----
            for ti, (s0, psz) in enumerate(((0, 128), (128, 72))):
                g0t = (ys_all[:, 0, :] if ti == 0 else ys_all[0:72, 1, :])
                g2t = (g2a if ti == 0 else g2b)[:psz]
                g1t = g1_t[ti]
                bf = ework.tile([128, 3], F32, tag="bf")
                nc.vector.tensor_copy(out=bf[:psz], in_=bi_t[ti][:psz])
                bh, bp, bt = bf[:psz, 0:1], bf[:psz, 1:2], bf[:psz, 2:3]

                def tt(o, i0, i1, op):
                    nc.vector.tensor_tensor(out=o, in0=i0, in1=i1, op=op)

                sc = ework.tile([128, 16], F32, tag="sc")
                c0 = sc[:psz, 0:1]
                c1 = sc[:psz, 1:2]
                s_sd = sc[:psz, 2:3]
                score = sc[:psz, 3:4]
                alpha = sc[:psz, 4:5]
                d2 = sc[:psz, 5:6]
                sig = sc[:psz, 6:7]
                q = sc[:psz, 7:8]
                aa = sc[:psz, 8:9]
                tmp = sc[:psz, 9:10]
                oma = sc[:psz, 10:11]

                tt(alpha, bp, bh, OP.subtract)          # bp - bh
                tt(d2, bt, bh, OP.subtract)             # bt - bh
                nc.vector.reciprocal(out=d2, in_=d2)
                tt(alpha, alpha, d2, OP.mult)           # alpha
                tt(sig, bt, bp, OP.subtract)            # bt - bp
                tt(sig, alpha, sig, OP.mult)            # sigma
                tt(sig, sig, sig, OP.mult)              # sigma^2
                nc.vector.reciprocal(out=c1, in_=sig)   # c1 = 1/sigma^2

                a_t = ework.tile([128, PROJ], BF16, tag="a")
                prod = ework.tile([128, PROJ], F32, tag="prod")
                nc.vector.tensor_scalar(out=oma, in0=alpha, scalar1=-1.0,
                                        scalar2=1.0, op0=OP.mult, op1=OP.add)
                nc.vector.tensor_scalar(out=prod[:psz], in0=g0t, scalar1=oma,
                                        scalar2=None, op0=OP.mult)
                nc.vector.scalar_tensor_tensor(
                    out=a_t[:psz], in0=g2t, scalar=alpha, in1=prod[:psz],
                    op0=OP.mult, op1=OP.add)

                # q = a.g1 ; aa = a.a ; score = g0.g2
                nc.vector.scalar_tensor_tensor(
                    out=prod[:psz], in0=a_t[:psz], scalar=1.0, in1=g1t[:psz],
                    op0=OP.mult, op1=OP.mult, accum_out=q)
                nc.vector.scalar_tensor_tensor(
                    out=prod[:psz], in0=a_t[:psz], scalar=1.0, in1=a_t[:psz],
                    op0=OP.mult, op1=OP.mult, accum_out=aa)
                nc.vector.scalar_tensor_tensor(
                    out=prod[:psz], in0=g0t, scalar=1.0, in1=g2t,
                    op0=OP.mult, op1=OP.mult, accum_out=score)

                # s = -(1 - 2q + aa) / (2 sigma^2)
                nc.vector.tensor_scalar(out=tmp, in0=q, scalar1=-2.0,
                                        scalar2=1.0, op0=OP.mult, op1=OP.add)
                tt(tmp, tmp, aa, OP.add)
                nc.vector.tensor_scalar(out=s_sd, in0=tmp, scalar1=c1,
                                        scalar2=-0.5, op0=OP.mult,
                                        op1=OP.mult)
                # c0 = -(1 + aa) / (2 sigma^2)
                nc.vector.tensor_scalar(out=tmp, in0=aa, scalar1=1.0,
                                        scalar2=None, op0=OP.add)
                nc.vector.tensor_scalar(out=c0, in0=tmp, scalar1=c1,
                                        scalar2=-0.5, op0=OP.mult,
                                        op1=OP.mult)

                nc.sync.dma_start(out=a_out[s0:s0 + psz, :], in_=a_t[:psz])
                nc.sync.dma_start(out=sc_out[s0:s0 + psz, :],
                                  in_=sc[:psz, 0:4])
    nc.compile()
    return nc


POOLN = 2 * NSEQ           # 3200 negative-pool columns (full pool)
CHUNK = 512                # psum chunk of pool columns
NCHK = (POOLN + CHUNK - 1) // CHUNK  # 7 chunks (6x512 + 1x128)


def _build_lb(mtb):
    nc = bacc.Bacc("TRN2", target_bir_lowering=False, debug=False,
                   num_devices=NCORES)
    a_in = nc.declare_dram_parameter("a_in", [HID, mtb * 128], FP8,
                                     isOutput=False)
    pool_in = nc.declare_dram_parameter("pool_in", [HID, mtb, POOLN], FP8,
                                        isOutput=False)
    scal = nc.declare_dram_parameter("scal", [128, mtb, 4], F32,
                                     isOutput=False)
    # scal cols: 0=c0, 1=c1, 2=s, 3=score; pads c0=c1=0, s=-1e4, score=+1e4
    out2 = nc.declare_dram_parameter("out2", [1, 2], F32, isOutput=True)

    with tile.TileContext(nc) as tc:
        with (
            tc.tile_pool(name="singles", bufs=1) as singles,
            tc.tile_pool(name="psum", bufs=7, space="PSUM") as psum_pool,
            tc.tile_pool(name="psum2", bufs=1, space="PSUM") as psum_pool2,
        ):
            engs = (nc.sync, nc.scalar)
            a_sb = []
            # a-vectors first (needed for the very first LDWEIGHTS), then
            # each tile's first pool chunk, then the pool remainders
            for kt in range(2):
                a_t = singles.tile([128, mtb * 128], FP8, tag=f"a{kt}")
                engs[kt].dma_start(out=a_t,
                                   in_=a_in[kt * 128:(kt + 1) * 128, :])
                a_sb.append(a_t)
            pool_sb = [[None] * mtb for _ in range(2)]
            pa = {}
            for g in range(mtb):
                for kt in range(2):
                    p_a = singles.tile([128, CHUNK], FP8, tag=f"pa{kt}_{g}")
                    engs[kt].dma_start(
                        out=p_a,
                        in_=pool_in[kt * 128:(kt + 1) * 128, g, :CHUNK])
                    pa[(kt, g)] = p_a
            for g in range(mtb):
                for kt in range(2):
                    p_b = singles.tile([128, POOLN - CHUNK], FP8,
                                       tag=f"pb{kt}_{g}")
                    rem = POOLN - CHUNK
                    qsz = rem // 3
                    for q in range(3):
                        q1 = rem if q == 2 else (q + 1) * qsz
                        engs[kt].dma_start(
                            out=p_b[:, q * qsz:q1],
                            in_=pool_in[kt * 128:(kt + 1) * 128, g,
                                        CHUNK + q * qsz:CHUNK + q1])
                    pool_sb[kt][g] = (pa[(kt, g)], p_b)
            sc_sb = singles.tile([128, mtb, 4], F32, tag="scal")
            nc.gpsimd.dma_start(out=sc_sb, in_=scal[:, :, :])

            # per M-tile: psum chunks of 512 pool columns, Max8 each, then
            # a merge Max8 over the chunk winners
            t8c = singles.tile([128, mtb, NCHK, 8], F32, tag="t8c")
            t8b = singles.tile([128, mtb, 8], F32, tag="t8b")
            for m in range(mtb):
                # one LDWEIGHTS per (m, kt): sweep all chunks on the same
                # stationary a-tile, accumulating across the two kt sweeps
                pss = [psum_pool.tile([128, CHUNK], F32, tag="ps",
                                      name=f"ps{m}_{c}")
                       for c in range(NCHK)]
                for kt in range(2):
                    for c in range(NCHK):
                        c0 = c * CHUNK
                        csz = min(CHUNK, POOLN - c0)
                        p_a, p_b = pool_sb[kt][m]
                        rhs = (p_a[:, :csz] if c == 0
                               else p_b[:, c0 - CHUNK:c0 - CHUNK + csz])
                        nc.tensor.matmul(
                            out=pss[c][:, :csz],
                            lhsT=a_sb[kt][:, m * 128:(m + 1) * 128],
                            rhs=rhs,
                            start=(kt == 0),
                            stop=(kt == 1),
                        )
                for c in range(NCHK):
                    c0 = c * CHUNK
                    csz = min(CHUNK, POOLN - c0)
                    nc.vector.max(out=t8c[:, m, c, :], in_=pss[c][:, :csz])
                nc.vector.max(out=t8b[:, m, :], in_=t8c[:, m, :, :])

            d8 = singles.tile([128, mtb, 8], F32, tag="d8")
            c1b = sc_sb[:, :, 1:2].to_broadcast([128, mtb, 8])
            c0b = sc_sb[:, :, 0:1].to_broadcast([128, mtb, 8])
            nc.vector.tensor_tensor(out=d8, in0=t8b, in1=c1b, op=OP.mult)
            nc.vector.tensor_tensor(out=d8, in0=d8, in1=c0b, op=OP.add)

            e6 = singles.tile([128, mtb, 6], F32, tag="e6")
            nc.scalar.activation(out=e6, in_=d8[:, :, 0:6], func=AF.Exp)
            se6 = singles.tile([128, mtb], F32, tag="se6")
            nc.vector.reduce_sum(out=se6[:, :].unsqueeze(-1), in_=e6,
                                 axis=mybir.AxisListType.X)
            numer = singles.tile([128, mtb], F32, tag="numer")
            nc.scalar.activation(out=numer[:, :].unsqueeze(-1),
                                 in_=sc_sb[:, :, 2:3], func=AF.Exp)
            # exp(max(v5, s)) = max(exp(v5), exp(s)) -- stays on DVE
            em = singles.tile([128, mtb], F32, tag="em")
            nc.vector.tensor_tensor(out=em[:, :].unsqueeze(-1),
                                    in0=e6[:, :, 5:6],
                                    in1=numer[:, :].unsqueeze(-1), op=OP.max)
            deno = singles.tile([128, mtb], F32, tag="deno")
            nc.vector.tensor_tensor(out=deno, in0=se6, in1=em, op=OP.subtract)
            nc.vector.tensor_tensor(out=deno, in0=deno, in1=numer, op=OP.add)
            nc.vector.reciprocal(out=deno, in_=deno)
            loss = singles.tile([128, mtb], F32, tag="loss")
            nc.vector.tensor_tensor(out=loss, in0=numer, in1=deno, op=OP.mult)

            # softplus(delta - score) = ln(1 + exp(delta - score))
            ones = singles.tile([128, 1], F32, tag="ones")
            nc.vector.memset(ones, 1.0)
            delta_sb = singles.tile([128, 1], F32, tag="delta")
            nc.vector.memset(delta_sb, DELTA)
            spt = singles.tile([128, mtb], F32, tag="spt")
            nc.scalar.activation(out=spt[:, :].unsqueeze(-1),
                                 in_=sc_sb[:, :, 3:4], func=AF.Exp,
                                 bias=delta_sb, scale=-1.0)
            sp = singles.tile([128, mtb], F32, tag="sp")
            nc.scalar.activation(out=sp, in_=spt, func=AF.Ln, bias=ones)

            red = singles.tile([128, 2], F32, tag="red")
            nc.vector.reduce_sum(out=red[:, 0:1], in_=loss,
                                 axis=mybir.AxisListType.X)
            nc.vector.reduce_sum(out=red[:, 1:2], in_=sp,
                                 axis=mybir.AxisListType.X)
            ps2 = psum_pool2.tile([1, 2], F32)
            nc.tensor.matmul(out=ps2, lhsT=ones[:, 0:1], rhs=red,
                             start=True, stop=True)
            fin = singles.tile([1, 2], F32, tag="fin")
            nc.vector.tensor_scalar(out=fin, in0=ps2, scalar1=1.0 / NSEQ,
                                    scalar2=None, op0=OP.mult)
            nc.sync.dma_start(out=out2[:, :], in_=fin)
    nc.compile()
    return nc


_NC_CACHE = {}


def _get(name, builder):
    if name not in _NC_CACHE:
        _NC_CACHE[name] = builder()
    return _NC_CACHE[name]


LAST_RUNS = []


def _hw_runner(nc, in_maps):
    import os
    res = run_bass_kernel_spmd(
        nc, in_maps, list(range(NCORES)),
        trace=bool(os.environ.get("KERNEL_TRACE")))
    LAST_RUNS.append(res)
    return res.results


def kernel(frame_embeds, other_frame_embeds, W, b, bridge, _runner=None):
    frame_embeds = np.asarray(frame_embeds, dtype=np.float32)
    other_frame_embeds = np.asarray(other_frame_embeds, dtype=np.float32)
    W = np.asarray(W, dtype=np.float32)
    b = np.asarray(b, dtype=np.float32)
    bridge = np.asarray(bridge, dtype=np.int32)

    runner = _runner if _runner is not None else _hw_runner

    # ---- host-side sharding / layout (pure indexing) ----
    fe_seq = frame_embeds.transpose(0, 2, 1, 3).reshape(NSEQ, T, HID)
    ofe_seq = other_frame_embeds.transpose(0, 2, 1, 3).reshape(NSEQ, T, HID)
    perm = np.argsort(bridge[:, 1], kind="stable")
    fe_sorted = fe_seq[perm]
    bridge_s = bridge[perm]

    w_bf = W.astype(NPBF)
    b2 = np.ascontiguousarray(
        np.tile(b.reshape(1, 1, PROJ), (128, 4, 1))).astype(NPBF)
    in_a = []
    for k in range(NCORES):
        sl = slice(k * SPC, (k + 1) * SPC)
        xt = np.zeros((HID, RPAD), dtype=NPBF)
        xt[:, :CUR_ROWS] = (
            fe_sorted[sl].transpose(2, 1, 0).reshape(HID, CUR_ROWS))
        xt[:, CUR_ROWS:ROWS] = (
            ofe_seq[sl, 1:T - 1].transpose(2, 1, 0).reshape(HID, OTH_ROWS))
        br_k = np.ascontiguousarray(bridge_s[sl])
        g1i = ((br_k[:, 1].astype(np.int32) - 1) * SPC
               + np.arange(SPC, dtype=np.int32)).reshape(SPC, 1)
        in_a.append({"xt_in": xt, "w_in": w_bf, "b2_in": b2,
                     "br_in": br_k, "g1idx": g1i})

    nca = _get("la", _build_la)
    ra = runner(nca, in_a)

    # ---- host reshard between launches (pure indexing) ----
    a_all = np.concatenate([ra[k]["a_out"] for k in range(NCORES)], axis=0)
    sc_all = np.concatenate([ra[k]["sc_out"] for k in range(NCORES)], axis=0)

    piv = bridge_s[:, 1].astype(np.int64)  # sorted ascending, values 1..14
    counts = np.bincount(piv, minlength=T)[1:T - 1]
    # per-group tile capacity: ceil(count/128) 128-slot M-tiles
    gtiles = [int(-(-int(c) // 128)) if c > 0 else 0 for c in counts]
    stiles = sum(gtiles)
    # (group, local tile) in group order; paired tiles share a core
    tlist = []
    for g in range(NG):
        for _ in range(gtiles[g]):
            tlist.append(g)
    mtb = max(2, -(-stiles // NCORES))
    nslots = NCORES * mtb * 128

    gstart = np.zeros(NG, dtype=np.int64)
    acc = 0
    for g in range(NG):
        gstart[g] = acc
        acc += gtiles[g] * 128
    rankpos = np.zeros(NSEQ, dtype=np.int64)
    rr = np.zeros(NG, dtype=np.int64)
    for i in range(NSEQ):
        g = piv[i] - 1
        rankpos[i] = gstart[g] + rr[g]
        rr[g] += 1
    slot_of = rankpos  # slot per sorted row, < stiles*128

    a_pad = np.zeros((HID, nslots), dtype=NPF8)
    a_pad[:, slot_of] = a_all.T

    scal = np.zeros((128, NCORES * mtb, 4), dtype=np.float32)
    scal[:, :, 2] = -1.0e4
    scal[:, :, 3] = 1.0e4
    scal[slot_of % 128, slot_of // 128, :] = sc_all

    # full negative pool per pivot group: columns = [cur sorted; oth],
    # built from the per-core emb outputs (pure indexing)
    embs = np.stack([ra[k]["emb"] for k in range(NCORES)])  # (8, 5600, 256)
    cur_part = embs[:, :POOL_CUR].reshape(NCORES, NG, SPC, HID)
    oth_part = embs[:, POOL_CUR:].reshape(NCORES, NG, SPC, HID)

    pool_of_g = {}

    def group_pool(g):
        if g not in pool_of_g:
            pg = np.empty((HID, POOLN), dtype=NPF8)
            pg[:, :NSEQ] = cur_part[:, g].reshape(POOLN // 2, HID).T
            pg[:, NSEQ:] = oth_part[:, g].reshape(POOLN // 2, HID).T
            pool_of_g[g] = pg
        return pool_of_g[g]

    in_b = []
    zero_pool = np.zeros((HID, POOLN), dtype=NPF8)
    for k in range(NCORES):
        tsel = [k * mtb + i for i in range(mtb)]
        pool_k = np.empty((HID, mtb, POOLN), dtype=NPF8)
        for gi, t in enumerate(tsel):
            if t < stiles:
                pool_k[:, gi, :] = group_pool(tlist[t])
            else:
                pool_k[:, gi, :] = zero_pool
        a_k = np.ascontiguousarray(
            a_pad[:, k * mtb * 128:(k + 1) * mtb * 128])
        scal_k = np.ascontiguousarray(scal[:, k * mtb:(k + 1) * mtb, :])
        in_b.append({"a_in": a_k, "pool_in": pool_k, "scal": scal_k})

    ncb = _get(("lb", mtb), lambda: _build_lb(mtb))
    rb = runner(ncb, in_b)

    parts = np.stack([rb[k]["out2"][0] for k in range(NCORES)])  # (8, 2)
    total = parts.sum(axis=0)
    brownian_loss = np.float32(total[0])
    head_tail_match = np.float32(total[1])
    return (np.asarray(brownian_loss), np.asarray(head_tail_match))
